# revision 1
# baseline (speedup 1.0000x reference)
"""Trainium2 Bass kernel for nn_NeuralRenderer (image_size=256, F=640 faces).

Strategy (per sharding hint): pixel rows sharded across 8 NeuronCores; faces /
textures replicated. Each core rasterizes its 8192-pixel band against all
faces:

  phase 1 (device): per-(pixel,face) barycentric planes w0,w1,w2 and depth are
    affine in (px,py,1) -> computed as a K=3 fp32 matmul on the PE into PSUM.
    ScalarE turns them into relu(-w) penalties (bf16), DVE folds them into a
    packed key  keyn = -depth - 1e34*penalty  and finds the per-pixel argmax
    over faces with the DVE max8/max_index ops (= nearest visible face).
  phase 2 (device): winner's face record (vertex/edge/det/z data, 48B) is
    gathered with indirect DMA; barycentrics are recomputed with the exact
    f32 operation order of the reference (division via bit-exact reciprocal),
    validity uses exact sign tests; texel rows (12B) are gathered from the
    pre-lit tanh'd texture table by indirect DMA; shading + masking on DVE.

Host does only the O(F) per-face setup (projection, affine coefficients,
texture table prep) plus input sharding / output concat.
"""

import numpy as np

IMG = 256
F = 640
NCORES = 8
PPC = IMG * IMG // NCORES    # pixels per core = 8192
NT = PPC // 128              # pixel tiles per core = 64
NREC = 12                    # face record floats
TEXROWS = F * 216
USE_FP32R = True             # PE fp32 "replicated" fast path (1 cyc/col)

_CACHE: dict = {}


# ----------------------------------------------------------------------------
# Device program
# ----------------------------------------------------------------------------

def _build_program(debug=False):
    import concourse.bass as bass
    import concourse.bacc as bacc
    import concourse.mybir as mybir
    import concourse.tile as tile

    dt = mybir.dt
    Alu = mybir.AluOpType
    Act = mybir.ActivationFunctionType

    nc = bacc.Bacc(None, target_bir_lowering=False)
    dbg = {}
    if debug:
        dbg["psA"] = nc.dram_tensor("dbg_psA", [128, 1280], dt.float32, kind="ExternalOutput")
        dbg["keyn"] = nc.dram_tensor("dbg_keyn", [128, 640], dt.float32, kind="ExternalOutput")
        dbg["m8"] = nc.dram_tensor("dbg_m8", [128, NT, 8], dt.float32, kind="ExternalOutput")
        dbg["i8"] = nc.dram_tensor("dbg_i8", [128, NT, 8], dt.uint32, kind="ExternalOutput")
        dbg["crec"] = nc.dram_tensor("dbg_crec", [128, NT, NREC], dt.float32, kind="ExternalOutput")
        dbg["vm"] = nc.dram_tensor("dbg_vm", [128, NT], dt.float32, kind="ExternalOutput")
        dbg["flat"] = nc.dram_tensor("dbg_flat", [128, NT], dt.int32, kind="ExternalOutput")
        dbg["ctex"] = nc.dram_tensor("dbg_ctex", [128, NT, 3], dt.float32, kind="ExternalOutput")
        dbg["b0"] = nc.dram_tensor("dbg_b0", [128, NT], dt.float32, kind="ExternalOutput")
        dbg["echo"] = nc.dram_tensor("dbg_echo", [128, NT], dt.float32, kind="ExternalOutput")
    pxT_d = nc.dram_tensor("pxT", [3, PPC], dt.float16, kind="ExternalInput")
    pxy_d = nc.dram_tensor("pxy", [2, 128, NT], dt.float32, kind="ExternalInput")
    faceBh_d = nc.dram_tensor("faceBh", [3, 4 * F], dt.float16, kind="ExternalInput")
    faceBl_d = nc.dram_tensor("faceBl", [3, 4 * F], dt.float16, kind="ExternalInput")
    frec_d = nc.dram_tensor("frec", [F, NREC], dt.float32, kind="ExternalInput")
    texlit_d = nc.dram_tensor("texlit", [TEXROWS, 3], dt.float32, kind="ExternalInput")
    img_d = nc.dram_tensor("img", [3, 128, NT], dt.float32, kind="ExternalOutput")

    # matmul output segments within a [128, 1280] (2-plane) PSUM tile, each
    # inside a single 512-f32 PSUM bank (bank-aligned, 3 matmuls per half)
    segs = [(0, 512), (512, 1024), (1024, 1280)]

    with tile.TileContext(nc) as tc:
        with (
            tc.tile_pool(name="const", bufs=1) as cp,
            tc.tile_pool(name="work", bufs=4) as wp,
            tc.tile_pool(name="p2", bufs=1) as p2,
            tc.tile_pool(name="psA", bufs=3, space="PSUM") as ppA,
            tc.tile_pool(name="psB", bufs=2, space="PSUM") as ppB,
        ):
            pxT = cp.tile([3, PPC], dt.float16)
            nc.sync.dma_start(pxT[:], pxT_d[:])
            faceBh = cp.tile([3, 4 * F], dt.float16)
            nc.sync.dma_start(faceBh[:], faceBh_d[:])
            faceBl = cp.tile([3, 4 * F], dt.float16)
            nc.sync.dma_start(faceBl[:], faceBl_d[:])
            pxv = cp.tile([128, NT], dt.float32)
            nc.sync.dma_start(pxv[:], pxy_d[0])
            pyv = cp.tile([128, NT], dt.float32)
            nc.sync.dma_start(pyv[:], pxy_d[1])
            m8buf = cp.tile([128, NT, 8], dt.float32)
            i8buf = cp.tile([128, NT, 8], dt.uint32)
            crec = cp.tile([128, NT, NREC], dt.float32)

            # ---------------- phase 1: winner face per pixel ----------------
            for t in range(NT):
                lhsT = pxT[:, t * 128:(t + 1) * 128]
                # plane columns [w0|w1|w2|d] (4*640) split into bank-aligned
                # psum tiles: 1024 + 1024 + 512 so the PE can run ahead of
                # the ScalarE drains (3+2 slots in flight)
                T0 = ppA.tile([128, 1024], dt.float32, tag="pA")
                T1 = ppA.tile([128, 1024], dt.float32, tag="pA")
                T2 = ppB.tile([128, 512], dt.float32, tag="pB")
                for ps, base, width in ((T0, 0, 1024), (T1, 1024, 1024), (T2, 2048, 512)):
                    for s in range(0, width, 512):
                        e = min(s + 512, width)
                        nc.tensor.matmul(ps[:, s:e], lhsT,
                                         faceBh[:, base + s:base + e],
                                         start=True, stop=False)
                        nc.tensor.matmul(ps[:, s:e], lhsT,
                                         faceBl[:, base + s:base + e],
                                         start=False, stop=True)

                # relu(-x) of ALL planes (d's relu is the rD penalty term)
                rAll = wp.tile([128, 2560], dt.bfloat16, tag="rAll")
                nc.scalar.activation(rAll[:, 0:1024], T0[:], Act.Relu, scale=-1.0)
                nc.scalar.activation(rAll[:, 1024:2048], T1[:], Act.Relu, scale=-1.0)
                nc.scalar.activation(rAll[:, 2048:2560], T2[:], Act.Relu, scale=-1.0)
                dneg = wp.tile([128, 640], dt.float32, tag="dneg")
                nc.scalar.activation(dneg[:, 0:128], T1[:, 896:1024], Act.Copy, scale=-1.0)
                nc.scalar.activation(dneg[:, 128:640], T2[:], Act.Copy, scale=-1.0)

                pen0 = wp.tile([128, 640], dt.bfloat16, tag="pen0")
                nc.vector.tensor_tensor(pen0[:], rAll[:, 0:640], rAll[:, 640:1280], op=Alu.add)
                pen1 = wp.tile([128, 640], dt.bfloat16, tag="pen1")
                nc.gpsimd.tensor_tensor(pen1[:], rAll[:, 1280:1920], rAll[:, 1920:2560], op=Alu.add)
                pen2 = wp.tile([128, 640], dt.bfloat16, tag="pen2")
                nc.vector.tensor_tensor(pen2[:], pen0[:], pen1[:], op=Alu.add)
                keyn = wp.tile([128, 640], dt.float32, tag="keyn")
                nc.vector.scalar_tensor_tensor(
                    keyn[:], pen2[:], -1e34, dneg[:],
                    op0=Alu.mult, op1=Alu.add)

                nc.vector.max(m8buf[:, t], keyn[:])
                nc.vector.max_index(i8buf[:, t], m8buf[:, t], keyn[:])
                # winner record gather (one row per partition — the only
                # indirect-DMA shape that matches hardware semantics);
                # issued per tile so it overlaps the raster loop
                nc.gpsimd.indirect_dma_start(
                    out=crec[:, t], out_offset=None, in_=frec_d[:],
                    in_offset=bass.IndirectOffsetOnAxis(
                        ap=i8buf[:, t, 0:1], axis=0))
                if debug and t == 0:
                    psAc = wp.tile([128, 1024], dt.float32, tag="psAc")
                    nc.scalar.activation(psAc[:], T0[:], Act.Copy)
                    nc.sync.dma_start(dbg["psA"][:, 0:1024], psAc[:])
                    nc.sync.dma_start(dbg["keyn"][:], keyn[:])

            # ---------------- phase 2: exact recompute + shading ------------
            def tt(name, in0, in1, op, dtype=dt.float32):
                o = p2.tile([128, NT], dtype, tag=name)
                nc.vector.tensor_tensor(o[:], in0, in1, op=op)
                return o

            def ts(name, in0, s1, s2, op0, op1=None, dtype=dt.float32):
                o = p2.tile([128, NT], dtype, tag=name)
                if op1 is None:
                    nc.vector.tensor_scalar(o[:], in0, s1, None, op0=op0)
                else:
                    nc.vector.tensor_scalar(o[:], in0, s1, s2, op0=op0, op1=op1)
                return o

            ch = lambda k: crec[:, :, k]
            # record: 0:x0 1:y0 2:d0 3:d1 4:e0 5:e1 6:det_s 7:detok
            #         8:z0 9:z1 10:z2 11:texbase(=216*face)

            qx = tt("qx", pxv[:], ch(0), Alu.subtract)
            qy = tt("qy", pyv[:], ch(1), Alu.subtract)
            t1 = tt("t1", qx[:], ch(5), Alu.mult)
            t2 = tt("t2", qy[:], ch(4), Alu.mult)
            n1 = tt("n1", t1[:], t2[:], Alu.subtract)
            t3 = tt("t3", ch(2), qy[:], Alu.mult)
            t4 = tt("t4", ch(3), qx[:], Alu.mult)
            n2 = tt("n2", t3[:], t4[:], Alu.subtract)
            rdet = p2.tile([128, NT], dt.float32)
            nc.vector.reciprocal(rdet[:], ch(6))
            b1 = tt("b1", n1[:], rdet[:], Alu.mult)
            b2 = tt("b2", n2[:], rdet[:], Alu.mult)
            u = ts("u", b1[:], -1.0, 1.0, Alu.mult, Alu.add)       # 1 - b1
            b0 = tt("b0", u[:], b2[:], Alu.subtract)

            # validity: exact sign tests for b1,b2 (sign(n/det) == sign(n*det))
            s1v = tt("s1v", n1[:], ch(6), Alu.mult)
            g1 = ts("g1", s1v[:], 0.0, None, Alu.is_ge)
            s2v = tt("s2v", n2[:], ch(6), Alu.mult)
            g2 = ts("g2", s2v[:], 0.0, None, Alu.is_ge)
            g0 = ts("g0", b0[:], 0.0, None, Alu.is_ge)
            m0 = tt("m0", b0[:], ch(8), Alu.mult)
            m1 = tt("m1", b1[:], ch(9), Alu.mult)
            s01 = tt("s01", m0[:], m1[:], Alu.add)
            m2 = tt("m2", b2[:], ch(10), Alu.mult)
            dw = tt("dw", s01[:], m2[:], Alu.add)
            gd = ts("gd", dw[:], 0.0, None, Alu.is_gt)
            vm = tt("vm", g1[:], g2[:], Alu.mult)
            vm = tt("vmb", vm[:], g0[:], Alu.mult)
            vm = tt("vmc", vm[:], gd[:], Alu.mult)
            vm = tt("vmd", vm[:], ch(7), Alu.mult)

            # texture cell indices: floor(clip(6*b, 0, 5.5)) robust to the
            # hardware's f32->int rounding mode (round-trip + correction)
            def floor_idx(name, b):
                x = ts(name + "x", b[:], 6.0, 0.0, Alu.mult, Alu.max)
                x = ts(name + "c", x[:], 5.5, None, Alu.min)
                ji = p2.tile([128, NT], dt.int32, tag=name + "i")
                nc.vector.tensor_copy(ji[:], x[:])
                jf = p2.tile([128, NT], dt.float32, tag=name + "f")
                nc.vector.tensor_copy(jf[:], ji[:])
                gt = tt(name + "g", jf[:], x[:], Alu.is_gt)
                return tt(name + "r", jf[:], gt[:], Alu.subtract)

            i0f = floor_idx("i0", b0)
            i1f = floor_idx("i1", b1)
            i2f = floor_idx("i2", b2)
            ffb = p2.tile([128, NT], dt.float32)
            nc.vector.scalar_tensor_tensor(ffb[:], i0f[:], 36.0, ch(11),
                                           op0=Alu.mult, op1=Alu.add)
            ffc = p2.tile([128, NT], dt.float32)
            nc.vector.scalar_tensor_tensor(ffc[:], i1f[:], 6.0, ffb[:],
                                           op0=Alu.mult, op1=Alu.add)
            ffd = tt("ffd", ffc[:], i2f[:], Alu.add)
            flat = p2.tile([128, NT], dt.int32)
            nc.vector.tensor_copy(flat[:], ffd[:])

            ctex = p2.tile([128, NT, 3], dt.float32)
            for t in range(NT):
                nc.gpsimd.indirect_dma_start(
                    out=ctex[:, t], out_offset=None, in_=texlit_d[:],
                    in_offset=bass.IndirectOffsetOnAxis(
                        ap=flat[:, t:t + 1], axis=0))

            for c in range(3):
                outp = p2.tile([128, NT], dt.float32, tag=f"outp{c}")
                nc.vector.tensor_tensor(outp[:], ctex[:, :, c], vm[:], op=Alu.mult)
                nc.sync.dma_start(img_d[c], outp[:])

            if debug:
                nc.sync.dma_start(dbg["m8"][:], m8buf[:])
                nc.sync.dma_start(dbg["i8"][:], i8buf[:])
                nc.sync.dma_start(dbg["crec"][:], crec[:])
                nc.sync.dma_start(dbg["vm"][:], vm[:])
                nc.sync.dma_start(dbg["flat"][:], flat[:])
                nc.sync.dma_start(dbg["ctex"][:], ctex[:])
                nc.sync.dma_start(dbg["b0"][:], b0[:])
                nc.sync.dma_start(dbg["echo"][:], pxv[:])

    nc.compile()
    return nc


def _get_program(debug=False):
    key = ("nc", debug)
    if key not in _CACHE:
        _CACHE[key] = _build_program(debug)
    return _CACHE[key]


# ----------------------------------------------------------------------------
# Host-side per-face setup (O(F) work, mirrors reference f32 op order)
# ----------------------------------------------------------------------------

def _host_prep(vertices, faces, textures):
    f32 = np.float32
    v = np.asarray(vertices[0], f32)                        # [N,3]
    f = np.asarray(faces[0]).astype(np.int64)               # [F,3]
    fv = v[f]                                               # [F,3,3]

    # lighting (ambient 0.5 + directional 0.5 * relu(n.[0,0,1]))
    n = np.cross(fv[:, 1] - fv[:, 0], fv[:, 2] - fv[:, 0]).astype(f32)
    nrm = np.linalg.norm(n, axis=-1, keepdims=True).astype(f32)
    n = (n / (nrm + f32(1e-8))).astype(f32)
    light = (f32(0.5) + f32(0.5) * np.maximum(n[:, 2], f32(0.0))).astype(f32)

    # 'look' camera: R == I for these constants; eye (0,0,-2)
    vc = (fv - np.array([0.0, 0.0, -2.0], f32)).astype(f32)
    zc = vc[..., 2].astype(f32)                             # [F,3]
    wfov = f32(np.tan(np.deg2rad(f32(45.0), dtype=f32), dtype=f32))
    xy = (vc[..., :2] / (zc[..., None] * wfov + f32(1e-8))).astype(f32)

    v0 = xy[:, 0]
    dd = (xy[:, 1] - v0).astype(f32)                        # [F,2]
    ee = (xy[:, 2] - v0).astype(f32)                        # [F,2]
    det = (dd[:, 0] * ee[:, 1] - dd[:, 1] * ee[:, 0]).astype(f32)
    det_ok = np.abs(det) > f32(1e-8)
    det_s = np.where(det_ok, det, f32(1.0)).astype(f32)

    # affine coefficients (f64 for accuracy, cast f32)
    x0 = v0[:, 0].astype(np.float64); y0 = v0[:, 1].astype(np.float64)
    d0 = dd[:, 0].astype(np.float64); d1 = dd[:, 1].astype(np.float64)
    e0 = ee[:, 0].astype(np.float64); e1 = ee[:, 1].astype(np.float64)
    ds = det_s.astype(np.float64)
    a1 = np.stack([e1, -e0, e0 * y0 - e1 * x0], -1) / ds[:, None]
    a2 = np.stack([-d1, d0, d1 * x0 - d0 * y0], -1) / ds[:, None]
    a0 = -a1 - a2
    a0[:, 2] += 1.0
    zc64 = zc.astype(np.float64)
    ad = a0 * zc64[:, 0:1] + a1 * zc64[:, 1:2] + a2 * zc64[:, 2:3]

    bad = (~det_ok | ~np.isfinite(a0).all(1) | ~np.isfinite(a1).all(1)
           | ~np.isfinite(a2).all(1) | ~np.isfinite(ad).all(1))
    for a in (a0, a1, a2):
        a[bad] = np.array([0.0, 0.0, -1.0])
    ad[bad] = np.array([0.0, 0.0, 1.0])

    faceB = np.concatenate(
        [a0.T, a1.T, a2.T, ad.T], axis=1).astype(f32)       # [3, 4F]
    faceBh = faceB.astype(np.float16)
    faceBl = (faceB - faceBh.astype(f32)).astype(np.float16)

    frec = np.stack([
        v0[:, 0], v0[:, 1], dd[:, 0], dd[:, 1], ee[:, 0], ee[:, 1],
        det_s, det_ok.astype(f32), zc[:, 0], zc[:, 1], zc[:, 2],
        (np.arange(F) * 216).astype(f32)], -1).astype(f32)  # [F, 12]

    tex = np.tanh(np.asarray(textures[0], f32)).astype(f32)     # [F,6,6,6,3]
    texlit = (tex * light[:, None, None, None, None]).astype(f32)
    texlit = texlit.reshape(TEXROWS, 3)

    return faceBh, faceBl, frec, texlit


def _pixel_buffers():
    f32 = np.float32
    ps = ((np.arange(IMG, dtype=f32) + f32(0.5)) / f32(IMG) * f32(2.0)
          - f32(1.0))
    nps = (-ps).astype(f32)
    j = np.arange(128)
    t = np.arange(NT)
    bufs = []
    for c in range(NCORES):
        g = c * PPC + j[:, None] * NT + t[None, :]          # [128, NT]
        px = ps[g % IMG].astype(f32)
        py = nps[g // IMG].astype(f32)
        pxy = np.stack([px, py]).astype(f32)                # [2, 128, NT]
        M = np.empty((3, NT, 128), f32)
        M[0] = px.T
        M[1] = py.T
        M[2] = 1.0
        pxT = M.reshape(3, PPC).astype(np.float16)          # col t*128+j; exact
        bufs.append((pxT, pxy))
    return bufs


# ----------------------------------------------------------------------------
# Entry point
# ----------------------------------------------------------------------------

def _run(inputs, trace=False):
    from concourse.bass_utils import run_bass_kernel_spmd

    faceBh, faceBl, frec, texlit = _host_prep(
        np.asarray(inputs["vertices"]),
        np.asarray(inputs["faces"]),
        np.asarray(inputs["textures"]))
    nc = _get_program()
    in_maps = []
    for (pxT, pxy) in _pixel_buffers():
        in_maps.append({
            "pxT": pxT, "pxy": pxy, "faceBh": faceBh, "faceBl": faceBl,
            "frec": frec, "texlit": texlit,
        })
    res = run_bass_kernel_spmd(nc, in_maps, list(range(NCORES)), trace=trace)
    outs = [np.asarray(res.results[c]["img"]).reshape(3, PPC)
            for c in range(NCORES)]
    full = np.concatenate(outs, axis=1).reshape(3, IMG, IMG)[None]
    return full.astype(np.float32), res


def kernel(**inputs) -> np.ndarray:
    out, _ = _run(inputs, trace=False)
    return out

